# revision 16
# baseline (speedup 1.0000x reference)
"""Multi-head attention (B=8, T=1024, D=768, H=12) on 8 TRN2 NeuronCores.

Sharding: data-parallel over batch — one batch element per core, no
collectives. Each core runs the full attention block for its element.

Per-core layout (feature-major activations; host pre-transposes):
  xT      [768, 1024]  bf16   (d-major activations)
  wqk     [768, 1536]  bf16   (qkv_w.T, q|k columns)
  wv      [768, 768]   bf16   (qkv_w.T, v columns)
  projT   [768, 768]   bf16   (proj_w.T)

Phase A (PE continuous, ACT idle): V = xT.T @ wv (token-major, ones
column per head appended for softmax denominators), then all 12 Q/K
projection tiles QKT[j,t] = wqk.T @ xT; bias folded in on DVE during
the PSUM->SBUF evacuation (cast to bf16).

Phase B (ACT-paced): per head pair, h-major flat loop over (hh, tk):
  scoresT[tk,tq]: lhsT=K.T-slice [64,128], rhs=Q.T [64,512] x2
  at = exp(scores * SCALE) on ACT   (no max subtraction; |s*scale|<~6)
  outT[hd+1, tq] = V'.T @ attnT accumulated in PSUM; row 64 = denom
  normalize per hh right after its last attnV (DVE recip + gpsimd
  partition_broadcast + DVE mul) so the oacc slot frees one half-pair
  early and the next pair never stalls on it.

Tail: yT = projT.T @ aoT (+bias'), ko ordered so pairs 0-4 accumulate
while pair 5 normalizes; bias' = proj_b + proj_w @ v_bias.
"""

import numpy as np
import ml_dtypes

import concourse.bass as bass
import concourse.mybir as mybir
import concourse.tile as tile
from concourse import bacc
from concourse import bass_utils

BF16 = mybir.dt.bfloat16
F32 = mybir.dt.float32

B, T, D = 8, 1024, 768
H, HD = 12, 64
P = 128
ND = D // P           # 6 d-tiles
NT = T // P           # 8 t-tiles
NPAIR = H // 2        # 6 head pairs
SCALE = HD ** -0.5
HD1 = HD + 1          # V' columns per head (64 V + 1 ones)


def build():
    nc = bacc.Bacc("TRN2", target_bir_lowering=False, debug=False, num_devices=8)

    xT_d = nc.dram_tensor("xT", [D, T], BF16, kind="ExternalInput").ap()
    wqk_d = nc.dram_tensor("wqk", [D, 2 * D], BF16, kind="ExternalInput").ap()
    wv_d = nc.dram_tensor("wv", [D, D], BF16, kind="ExternalInput").ap()
    projT_d = nc.dram_tensor("projT", [D, D], BF16, kind="ExternalInput").ap()
    qkb_d = nc.dram_tensor("qkb", [P, 2 * ND], F32, kind="ExternalInput").ap()
    pb2_d = nc.dram_tensor("pb2", [P, ND], F32, kind="ExternalInput").ap()
    yT_d = nc.dram_tensor("yT", [D, T], F32, kind="ExternalOutput").ap()

    with tile.TileContext(nc) as tc:
        with tc.tile_pool(name="const", bufs=1) as const, \
             tc.tile_pool(name="work", bufs=4) as work, \
             tc.tile_pool(name="norm", bufs=2) as normp, \
             tc.tile_pool(name="yout", bufs=2) as yout, \
             tc.tile_pool(name="psA", bufs=2, space="PSUM") as psA, \
             tc.tile_pool(name="psO", bufs=2, space="PSUM") as psO:

            # ---- resident SBUF tensors ----
            xT_sb = const.tile([P, ND, T], BF16)
            wv_sb = const.tile([P, ND, D], BF16)
            wqk_sb = const.tile([P, ND, 2 * D], BF16)
            projT_sb = const.tile([P, ND, D], BF16)
            qkb_sb = const.tile([P, 2 * ND], F32)
            pb2_sb = const.tile([P, ND], F32)
            QKT_sb = const.tile([P, 2 * ND, T], BF16)
            V_sb = const.tile([P, NT, H * HD1], BF16)
            # one aoT tile per pair so proj's per-ko matmuls only dep on
            # the pairs they actually read (granular RAW tracking)
            aoT_sb = [const.tile([P, T], BF16, name=f"aoT_{i}") for i in range(ND)]

            # first ko-chunk of xT and wv as standalone transfers so the
            # first V matmul can start as soon as ~0.46 MB lands, not the
            # whole 6 MB preamble
            xT_r = xT_d.rearrange("(ko p) t -> p ko t", p=P)
            wv_r = wv_d.rearrange("(ko p) j -> p ko j", p=P)
            nc.sync.dma_start(xT_sb[:, 0:1, :], xT_r[:, 0:1, :])
            nc.sync.dma_start(wv_sb[:, 0:1, :], wv_r[:, 0:1, :])
            for c in (1, 2):
                nc.sync.dma_start(xT_sb[:, c:c + 1, :], xT_r[:, c:c + 1, :])
                nc.sync.dma_start(wv_sb[:, c:c + 1, :], wv_r[:, c:c + 1, :])
            for c in (3, 4, 5):
                nc.sync.dma_start(xT_sb[:, c:c + 1, :], xT_r[:, c:c + 1, :])
                nc.sync.dma_start(wv_sb[:, c:c + 1, :], wv_r[:, c:c + 1, :])
            # wqk: j-tiles 0 and 6 (pair 0's Q and K weights) land first as
            # small transfers; the rest in two bulk strided transfers
            wqk_r = wqk_d.rearrange("(ko p) j -> p ko j", p=P)
            for lo, hi in ((0, P), (ND * P, (ND + 1) * P),
                           (P, ND * P), ((ND + 1) * P, 2 * D)):
                nc.sync.dma_start(wqk_sb[:, :, lo:hi], wqk_r[:, :, lo:hi])
            nc.sync.dma_start(qkb_sb[:], qkb_d)

            # ones columns for the softmax denominator (col 64 of each head's V')
            nc.vector.memset(V_sb[:], 1.0)

            # warm the exp table set early (one-time ~2.7us table load)
            warm = work.tile([1, 12], F32, tag="warm", bufs=1)
            nc.scalar.activation(warm[:], qkb_sb[0:1, 0:12], mybir.ActivationFunctionType.Exp)

            # ~5us of garbage matmuls at t=0 (QKT_sb is not DMA'd, so no WAR
            # on the input transfers; results land in a dead psO slot). They
            # warm the HAM clock gate during the initial DMA wait so the V
            # projection runs at 2.4 GHz instead of 1.2 from the start.
            ps_w = psO.tile([P, T], F32, tag="oacc", name="ps_warm")
            for w in range(12):
                nc.tensor.matmul(
                    ps_w[0:1, 0:512],
                    QKT_sb[:, 0, 0:1], QKT_sb[:, 0, 0:512],
                    skip_group_check=True)

            def emit_v(t, jc):
                # V[t, jv] = xT.T @ wv  (token-major), one (t-tile, j-chunk)
                j0, jn = [(0, 512), (512, 256)][jc]
                ps_v = psA.tile([P, T], F32, tag="big", name=f"psv_{t}_{jc}")
                for d in range(ND):
                    nc.tensor.matmul(
                        ps_v[:, :jn],
                        xT_sb[:, d, t * P:(t + 1) * P],
                        wv_sb[:, d, j0:j0 + jn],
                        start=(d == 0), stop=(d == ND - 1),
                    )
                nh = jn // HD
                h0 = j0 // HD
                dst = V_sb[:, t, :].rearrange("p (h c) -> p h c", c=HD1)
                nc.vector.tensor_copy(
                    out=dst[:, h0:h0 + nh, 0:HD],
                    in_=ps_v[:, :jn].rearrange("p (h c) -> p h c", c=HD),
                )

            def emit_qk(jt):
                # QKT[:, jt, :] for j-tile jt (0..5 = Q, 6..11 = K)
                ps_qk = psA.tile([P, T], F32, tag="big", name=f"psqk_{jt}")
                mm = None
                for tq in range(2):
                    for d in range(ND):
                        mm = nc.tensor.matmul(
                            ps_qk[:, tq * 512:(tq + 1) * 512],
                            wqk_sb[:, d, jt * P:(jt + 1) * P],
                            xT_sb[:, d, tq * 512:(tq + 1) * 512],
                            start=(d == 0), stop=(d == ND - 1),
                        )
                # bias-add + f32->bf16 cast on DVE during PSUM evacuation;
                # ACT stays exp-only in phase B
                nc.vector.tensor_scalar_add(
                    QKT_sb[:, jt, :], ps_qk[:], qkb_sb[:, jt:jt + 1])
                return mm

            # ---- phase A: V projection, then Q/K tiles ----
            for c in range(2 * NT):
                emit_v(c // 2, c % 2)
            anchor = emit_qk(0)
            # bulk weight DMAs (projT, pb2) gated behind the first QK tile so
            # the critical early loads get HBM bandwidth to themselves. They
            # go through the GpSimd software-DGE queues — putting a gated DMA
            # on the sync HWDGE queues can deadlock (FIFO inversion).
            from concourse.tile_rust import add_dep_helper
            bulk = [
                nc.gpsimd.dma_start(
                    projT_sb[:], projT_d.rearrange("(ko p) j -> p ko j", p=P)),
                nc.gpsimd.dma_start(pb2_sb[:], pb2_d),
            ]
            for b in bulk:
                add_dep_helper(b.ins, anchor.ins, sync=True,
                               reason="bulk weight DMA after first QK tile")
            # the last two QK tiles (jt=5, 11) are emitted inside pair 0's
            # prologue, interleaved with its first two scores/exp steps, so
            # the PE never idles across the phase A->B transition (a >~1us
            # idle there trips the HAM clock gate and the whole first pair
            # runs at 1.2 GHz)
            for jt in (ND, 1, ND + 1, 2, ND + 2, 3, ND + 3, 4, ND + 4):
                emit_qk(jt)

            # ---- phase B: attention, h-major flat step loop per pair ----
            # steps k = 0..15 map to (hh, tk); hh0's normalize runs during
            # hh1's steps so the oacc slot frees before the next pair needs it
            def emit_scores(i, k, at_tiles):
                hh, tk = divmod(k, NT)
                p0 = 64 * hh
                sc = psA.tile([P, T], F32, tag="big", name=f"sc_{i}_{k}")
                for tq in range(2):
                    nc.tensor.matmul(
                        sc[:, tq * 512:(tq + 1) * 512],
                        QKT_sb[p0:p0 + 64, ND + i, tk * P:(tk + 1) * P],
                        QKT_sb[p0:p0 + 64, i, tq * 512:(tq + 1) * 512],
                    )
                at = work.tile([P, T], BF16, tag="attn", name=f"at_{i}_{k}", bufs=8)
                nc.scalar.activation(
                    at[:], sc[:], mybir.ActivationFunctionType.Exp, scale=SCALE)
                at_tiles[k] = at

            def emit_attnv(i, k, oacc, at_tiles):
                hh, tk = divmod(k, NT)
                h = 2 * i + hh
                at = at_tiles.pop(k)
                for tq in range(2):
                    nc.tensor.matmul(
                        oacc[hh][:HD1, tq * 512:(tq + 1) * 512],
                        V_sb[:, tk, h * HD1:(h + 1) * HD1],
                        at[:, tq * 512:(tq + 1) * 512],
                        start=(tk == 0), stop=(tk == NT - 1),
                    )

            def emit_norm(i, hh, oacc_t, halves=1):
                # halves=2 splits the chain by tq so downstream proj matmuls
                # can start on the first half ~3us earlier (used on the very
                # last normalize, which gates the projection tail)
                w = T // halves
                for hf in range(halves):
                    sl = slice(hf * w, (hf + 1) * w)
                    den = normp.tile([1, w], F32, tag="den", name=f"den_{i}_{hh}_{hf}")
                    nc.vector.tensor_copy(out=den[:], in_=oacc_t[HD:HD1, sl])
                    rsb = normp.tile([1, w], F32, tag="rsb", name=f"rsb_{i}_{hh}_{hf}")
                    nc.vector.reciprocal_approx_fast(rsb[:], den[:])
                    rbc_sb = normp.tile([64, w], F32, tag="rbc", name=f"rbc_{i}_{hh}_{hf}")
                    nc.gpsimd.partition_broadcast(rbc_sb[:], rsb[:])
                    nc.vector.tensor_tensor(
                        aoT_sb[i][64 * hh:64 * hh + 64, sl],
                        oacc_t[0:HD, sl], rbc_sb[:], mybir.AluOpType.mult)

            # global flat step loop: the scores/exp pipeline runs LAG=2 steps
            # ahead continuously across pair boundaries, so ACT never waits a
            # fresh scores tile at a boundary
            LAG = 2
            NSTEP = 2 * NT * NPAIR
            at_tiles = {}
            oaccs = {}

            def sc_g(g, at_tiles=at_tiles):
                emit_scores(g // (2 * NT), g % (2 * NT), at_tiles)

            sc_g(0)
            sc_g(1)
            emit_qk(5)
            emit_qk(ND + 5)
            for g in range(NSTEP):
                i, k = divmod(g, 2 * NT)
                if k == 0:
                    oaccs[i] = [psO.tile([P, T], F32, tag="oacc", name=f"oacc_{i}_{hh}")
                                for hh in range(2)]
                if g + LAG < NSTEP:
                    sc_g(g + LAG)
                emit_attnv(i, k, oaccs[i], at_tiles)
                if k == NT - 1 + LAG:
                    # hh0's last attnV was 2 steps ago; normalize it now
                    emit_norm(i, 0, oaccs[i][0])
                if k == 2 * NT - 1:
                    emit_norm(i, 1, oaccs[i][1],
                              halves=(2 if i == NPAIR - 1 else 1))

            # ---- output projection: yT = projT.T @ aoT (+ pb2) ----
            # ko 0..5 are pairs 0..5. The ko 0..4 partials of the first two
            # dt tiles are emitted before their ko=5 matmuls so ~4us of proj
            # runs while pair 5's last normalize drains.
            def emit_proj_partial(dt, ps_y, kos, start, stop):
                for tq in range(2):
                    for n, ko in enumerate(kos):
                        nc.tensor.matmul(
                            ps_y[:, tq * 512:(tq + 1) * 512],
                            projT_sb[:, ko, dt * P:(dt + 1) * P],
                            aoT_sb[ko][:, tq * 512:(tq + 1) * 512],
                            start=start and (n == 0), stop=stop and (n == len(kos) - 1),
                        )

            ps_ys = {}
            for dt in (0, 1):
                ps_ys[dt] = psA.tile([P, T], F32, tag="big", name=f"psy_{dt}")
                emit_proj_partial(dt, ps_ys[dt], range(ND - 1), True, False)
            for dt in range(ND):
                if dt in ps_ys:
                    ps_y = ps_ys[dt]
                    emit_proj_partial(dt, ps_y, [ND - 1], False, True)
                else:
                    ps_y = psA.tile([P, T], F32, tag="big", name=f"psy_{dt}")
                    emit_proj_partial(dt, ps_y, range(ND), True, True)
                yt = yout.tile([P, T], F32, tag="yt", name=f"yt_{dt}")
                nc.vector.tensor_scalar_add(yt[:], ps_y[:], pb2_sb[:, dt:dt + 1])
                nc.sync.dma_start(yT_d[dt * P:(dt + 1) * P, :], yt[:])

    nc.compile()
    return nc


def prep_inputs(x, qkv_w, qkv_b, proj_w, proj_b):
    """Host-side layout prep. Returns per-core input maps."""
    bf = ml_dtypes.bfloat16
    wqkvT = np.ascontiguousarray(qkv_w.T)          # [768, 2304] f32
    wqk = wqkvT[:, :2 * D].astype(bf)
    wv = np.ascontiguousarray(wqkvT[:, 2 * D:]).astype(bf)
    projT = np.ascontiguousarray(proj_w.T).astype(bf)
    qkb = np.ascontiguousarray(
        qkv_b[:2 * D].reshape(2 * ND, P).T).astype(np.float32)   # [128, 12]
    vb = qkv_b[2 * D:]
    pb2 = (proj_b + proj_w @ vb).astype(np.float32)
    pb2 = np.ascontiguousarray(pb2.reshape(ND, P).T)             # [128, 6]

    in_maps = []
    for b in range(B):
        xT = np.ascontiguousarray(x[b].T).astype(bf)             # [768, 1024]
        in_maps.append({
            "xT": xT, "wqk": wqk, "wv": wv, "projT": projT,
            "qkb": qkb, "pb2": pb2,
        })
    return in_maps


_CACHE = {}


def kernel(x, qkv_w, qkv_b, proj_w, proj_b):
    x = np.asarray(x, dtype=np.float32)
    qkv_w = np.asarray(qkv_w, dtype=np.float32)
    qkv_b = np.asarray(qkv_b, dtype=np.float32)
    proj_w = np.asarray(proj_w, dtype=np.float32)
    proj_b = np.asarray(proj_b, dtype=np.float32)

    if "nc" not in _CACHE:
        _CACHE["nc"] = build()
    nc = _CACHE["nc"]

    in_maps = prep_inputs(x, qkv_w, qkv_b, proj_w, proj_b)
    res = bass_utils.run_bass_kernel_spmd(nc, in_maps, core_ids=list(range(8)))
    out = np.empty((B, T, D), np.float32)
    for b in range(B):
        out[b] = res.results[b]["yT"].T
    return out


if __name__ == "__main__":
    rng = np.random.default_rng(0)
    ins = {
        "x": rng.standard_normal((B, T, D), dtype=np.float32),
        "qkv_w": rng.standard_normal((3 * D, D), dtype=np.float32) * D ** -0.5,
        "qkv_b": rng.standard_normal(3 * D).astype(np.float32) * 0.02,
        "proj_w": rng.standard_normal((D, D), dtype=np.float32) * D ** -0.5,
        "proj_b": rng.standard_normal(D).astype(np.float32) * 0.02,
    }
    out = kernel(**ins)
    print("ok", out.shape, np.abs(out).max())
